# revision 10
# baseline (speedup 1.0000x reference)
"""Causal self-attention (B=2, T=2048, E=1024, H=16, D=64) on 8 TRN2 NeuronCores.

Sharding: core = (batch b, head-group hg): 2 batches x 4 head-groups of 4 heads.
Each core computes QKV projections for its 4 heads (256 columns), causal
attention, and the output projection against its 256 rows of Wo, producing a
partial [2048, 1024] fp16 output. Host sums the 4 head-group partials per
batch in fp32 (the tensor-parallel all-reduce) and adds bo.

Per-core kernel:
  - Q/K projections in fp8-e4m3 DoubleRow matmuls (contract 2x128 per step);
    K bias dropped entirely (softmax shift invariance), Q bias kept.
  - Q^T/K^T stored transposed fp8 with a zero second k-plane so the D=64
    score contraction also runs DoubleRow (0.5 cyc/row).
  - V projection and everything downstream in fp16 (fp8 V / attn weights
    would blow the 2e-2 error budget; fp8 upstream of softmax is fine).
  - attn@V FLIPPED: out[queries, D+1] = pt^T @ V per key tile, so the
    streamed output is only 65 wide (the cost model charges by output free
    size); causal trimming = simply not emitting invalid (key-tile, q-chunk)
    matmuls; the ones column appended to V puts the softmax row-sums on the
    QUERY partitions, so normalization is a per-partition reciprocal + mul
    (no cross-partition broadcast, no DRAM bounce).
  - A PE transpose (identity ifmap) per 128-query chunk rebuilds the
    [head-dims, queries] layout the fp16 output projection needs.
  - Stair-block causal masks (one 128-wide triangle per diagonal tile) run
    on GpSimd; exp stays on ScalarE, the overall bottleneck.
"""
from contextlib import ExitStack

import numpy as np
import ml_dtypes

import concourse.bass as bass  # noqa: F401
import concourse.mybir as mybir
import concourse.tile as tile
from concourse import bacc
from concourse.bass_utils import run_bass_kernel_spmd

T = 2048
E = 1024
HPC = 4          # heads per core
D = 64
S = HPC * D      # 256: per-core head-column slice
NKT = T // 128   # 16 key row tiles
NQB = T // 512   # 4 query column blocks
F8 = mybir.dt.float8e4
F16 = mybir.dt.float16
F32 = mybir.dt.float32
EXP = mybir.ActivationFunctionType.Exp
DR = mybir.MatmulPerfMode.DoubleRow
NP8 = ml_dtypes.float8_e4m3


def build_nc(phases=None):
    nc = bacc.Bacc("TRN2", target_bir_lowering=False, debug=False)
    x8 = nc.dram_tensor("x8", [E, T], F8, kind="ExternalInput").ap()
    xv = nc.dram_tensor("xv", [E, T], F16, kind="ExternalInput").ap()
    wq = nc.dram_tensor("wq", [E, S], F8, kind="ExternalInput").ap()
    wk = nc.dram_tensor("wk", [E, S], F8, kind="ExternalInput").ap()
    wv = nc.dram_tensor("wv", [E, S], F16, kind="ExternalInput").ap()
    wo = nc.dram_tensor("wo", [S, E], F16, kind="ExternalInput").ap()
    bq = nc.dram_tensor("bq", [S, 1], F32, kind="ExternalInput").ap()
    bv = nc.dram_tensor("bv", [1, S], F32, kind="ExternalInput").ap()
    tri = nc.dram_tensor("tri", [128, 256], F16, kind="ExternalInput").ap()
    eye = nc.dram_tensor("eye", [128, 128], F16, kind="ExternalInput").ap()
    out = nc.dram_tensor("out", [T, E], F16, kind="ExternalOutput").ap()

    with tile.TileContext(nc) as tc:
        _emit(nc, tc, x8, xv, wq, wk, wv, wo, bq, bv, tri, eye, out, phases=phases)
    nc.compile()
    return nc


def _emit(nc, tc, x8, xv, wq, wk, wv, wo, bq, bv, tri, eye, out, phases=None):
    ctx = ExitStack()
    consts = ctx.enter_context(tc.tile_pool(name="consts", bufs=1))
    mm_ps = ctx.enter_context(tc.tile_pool(name="mm_ps", bufs=2, space="PSUM"))
    st_ps = ctx.enter_context(tc.tile_pool(name="st_ps", bufs=2, space="PSUM"))
    ot_ps = ctx.enter_context(tc.tile_pool(name="ot_ps", bufs=2, space="PSUM"))
    pt_pool = ctx.enter_context(tc.tile_pool(name="pt", bufs=6))
    sm_pool = ctx.enter_context(tc.tile_pool(name="sm", bufs=8))

    # --- persistent SBUF tensors ---
    x_sb = consts.tile([128, 4, 2, T], F8)       # e = k2*256 + i*128 + p
    xv_sb = consts.tile([128, 8, T], F16)        # e = ke*128 + p
    wq_sb = consts.tile([128, 4, 2, S], F8)
    wk_sb = consts.tile([128, 4, 2, S], F8)
    wv_sb = consts.tile([128, 8, S], F16)
    wo_sb = consts.tile([128, S // 128, E], F16)
    bq_sb = consts.tile([128, 2], F32)
    bv_bc = consts.tile([128, S], F32)
    tri_sb = consts.tile([128, 2, 128], F16)     # stair mask, per-head copy
    eye_sb = consts.tile([128, 128], F16)
    qt_sb = consts.tile([128, 2, 2, T], F8)      # [hd dims, pair, k-plane, T]
    kt_sb = consts.tile([128, 2, 2, T], F8)
    v_sb = consts.tile([128, NKT, HPC, D + 1], F16)
    attnT_sb = consts.tile([128, 2, T], F16)

    # --- constant loads, ordered so the first QK group can start early ---
    xr = x8.rearrange("(k i p) n -> p k i n", p=128, i=2)
    xvr = xv.rearrange("(k p) n -> p k n", p=128)
    nc.sync.dma_start(out=wq_sb, in_=wq.rearrange("(k i p) m -> p k i m", p=128, i=2))
    nc.sync.dma_start(out=x_sb[:, :, :, 0:512], in_=xr[:, :, :, 0:512])
    nc.sync.dma_start(out=wk_sb, in_=wk.rearrange("(k i p) m -> p k i m", p=128, i=2))
    nc.sync.dma_start(out=wv_sb, in_=wv.rearrange("(k p) m -> p k m", p=128))
    nc.sync.dma_start(out=xv_sb[:, :, 0:512], in_=xvr[:, :, 0:512])
    nc.sync.dma_start(out=bq_sb, in_=bq.rearrange("(a p) one -> p (a one)", p=128))
    nc.sync.dma_start(out=bv_bc, in_=bv.to_broadcast((128, S)))
    nc.sync.dma_start(out=tri_sb, in_=tri.rearrange("p (a n) -> p a n", a=2))
    nc.sync.dma_start(out=eye_sb, in_=eye)
    for qb in range(1, NQB):
        qs = slice(qb * 512, (qb + 1) * 512)
        nc.sync.dma_start(out=x_sb[:, :, :, qs], in_=xr[:, :, :, qs])
        nc.sync.dma_start(out=xv_sb[:, :, qs], in_=xvr[:, :, qs])
    nc.sync.dma_start(out=wo_sb, in_=wo.rearrange("(a p) n -> p a n", p=128))
    # zero k-planes for the D=64 score contraction; ones column of V
    nc.gpsimd.memset(qt_sb[:, :, 1, :], 0.0)
    nc.gpsimd.memset(kt_sb[:, :, 1, :], 0.0)
    nc.gpsimd.memset(v_sb[:, :, :, D : D + 1], 1.0)

    # --- V = x @ wv + bv (fp16, with ones column) ---
    def emit_v(rts=range(NKT)):
        for rt in rts:
            ps = mm_ps.tile([128, 512], F32, tag="mm", name=f"vps{rt}")
            for ke in range(8):
                nc.tensor.matmul(
                    ps[:, 0:S],
                    lhsT=xv_sb[:, ke, rt * 128 : (rt + 1) * 128],
                    rhs=wv_sb[:, ke],
                    start=(ke == 0),
                    stop=(ke == 7),
                )
            nc.vector.tensor_add(
                v_sb[:, rt, :, 0:D],
                ps[:, 0:S].rearrange("p (h d) -> p h d", h=HPC),
                bv_bc.rearrange("p (h d) -> p h d", h=HPC),
            )

    # --- QT/KT = (x @ w [+ bq]).T fp8 for one pair of heads (128 cols) ---
    def emit_qk_part(p, qb):
        qs = slice(qb * 512, (qb + 1) * 512)
        for w_sb, dst, nm in ((wq_sb, qt_sb, "q"), (wk_sb, kt_sb, "k")):
            ps = mm_ps.tile([128, 512], F32, tag="mm", name=f"{nm}ps{p}_{qb}")
            for k2 in range(4):
                nc.tensor.matmul(
                    ps,
                    lhsT=w_sb[:, k2, :, p * 128 : (p + 1) * 128],
                    rhs=x_sb[:, k2, :, qs],
                    start=(k2 == 0),
                    stop=(k2 == 3),
                    perf_mode=DR,
                )
            if nm == "q":
                nc.vector.tensor_scalar_add(dst[:, p, 0, qs], ps, bq_sb[:, p : p + 1])
            else:
                nc.vector.tensor_copy(dst[:, p, 0, qs], ps)

    def emit_qk(p):
        for qb in range(NQB):
            emit_qk_part(p, qb)

    # --- attention for pair p (heads 2p, 2p+1), query block qb ---
    def emit_attn(p, qb, prenorm=()):
        nkt = 4 * (qb + 1)
        # one 2KB bank per head: 4 query-chunk units of 128 f32; a single
        # start=True matmul (kt=0, qc=0) zeroes the bank (PSUM zero-region),
        # everything else accumulates with start=False
        ots = [
            ot_ps.tile([128, 4, 128], F32, tag="ot", name=f"ot{p}_{qb}_{h}")
            for h in range(2)
        ]

        def do_st(kt):
            st = st_ps.tile([128, 1024], F32, tag="st", name=f"st{p}_{qb}_{kt}")
            for hh in range(2):
                hs = slice(hh * 64, (hh + 1) * 64)
                nc.tensor.matmul(
                    st[:, hh * 512 : (hh + 1) * 512],
                    lhsT=kt_sb[hs, p, :, kt * 128 : (kt + 1) * 128],
                    rhs=qt_sb[hs, p, :, qb * 512 : (qb + 1) * 512],
                    start=True,
                    stop=True,
                    perf_mode=DR,
                )
            return st

        # exp st -> pt16; diagonal tiles r>=1 trimmed to valid columns, the
        # 128-wide stair block masked with the triangle on GpSimd
        def do_exp(kt, st, buf):
            r = kt - 4 * qb if kt >= 4 * qb else None
            if r is None or r == 0:
                nc.scalar.activation(buf, st, EXP, scale=0.125)
            else:
                off = 128 * r
                stv = st.rearrange("p (a n) -> p a n", a=2)
                bfv = buf.rearrange("p (a n) -> p a n", a=2)
                nc.scalar.activation(bfv[:, :, off:512], stv[:, :, off:512], EXP, scale=0.125)
            if r is not None:
                off = 128 * r
                bfv = buf.rearrange("p (a n) -> p a n", a=2)
                nc.gpsimd.tensor_mul(
                    bfv[:, :, off : off + 128], bfv[:, :, off : off + 128], tri_sb
                )

        # software pipeline: scores one key tile ahead of exp; flipped attn@V
        # accumulates out[queries, D+1] per (key tile, head, query chunk)
        sts = {0: do_st(0)}
        for kt in range(nkt):
            if kt + 1 < nkt:
                sts[kt + 1] = do_st(kt + 1)
            buf = pt_pool.tile([128, 1024], F16, tag="pt", name=f"pt{p}_{qb}_{kt}")
            do_exp(kt, sts.pop(kt), buf)
            r = kt - 4 * qb if kt >= 4 * qb else None
            for hh in range(2):
                for qc in range(0 if r is None else r, 4):
                    nc.tensor.matmul(
                        ots[hh][:, qc, 0 : D + 1],
                        lhsT=buf[:, hh * 512 + qc * 128 : hh * 512 + (qc + 1) * 128],
                        rhs=v_sb[:, kt, 2 * p + hh, :],
                        start=(kt == 0 and qc == 0),
                        stop=(kt == 4 * qb + qc),
                        skip_group_check=True,
                    )

        # next block's projections go ahead of the norm ops in the DVE
        # stream so its scores aren't head-of-line blocked
        for hook in prenorm:
            hook()

        # normalization: per-partition (query) reciprocal of the row-sums,
        # then scale + fp16 convert; PE transpose restores [dims, queries]
        a16 = sm_pool.tile([128, 4, 128], F16, tag="a16", name=f"a16{p}_{qb}")
        for hh in range(2):
            rr = sm_pool.tile([128, 4], F32, tag="rr", name=f"rr{p}_{qb}_{hh}")
            nc.vector.reciprocal(rr, ots[hh][:, :, D : D + 1])
            nc.vector.tensor_mul(
                a16[:, :, hh * 64 : (hh + 1) * 64],
                ots[hh][:, :, 0:D],
                rr[:, :, None].to_broadcast((128, 4, D)),
            )
        for qc in range(4):
            tr = ot_ps.tile([128, 1024], F16, tag="ot", name=f"tr{p}_{qb}_{qc}")
            nc.tensor.transpose(tr[:, 0:128], a16[:, qc, :], eye_sb)
            qs = slice(qb * 512 + qc * 128, qb * 512 + (qc + 1) * 128)
            nc.vector.tensor_copy(attnT_sb[:, p, qs], tr[:, 0:128])

    # --- output projection: out tile = attnT.T @ wo (fp16) ---
    def emit_wo_part(qts):
        for qt in qts:
            o_sb = sm_pool.tile([128, 1024], F16, tag="ob", name=f"ob{qt}")
            for nt in range(2):
                ps = mm_ps.tile([128, 512], F32, tag="mm", name=f"ops{qt}_{nt}")
                for p in range(2):
                    nc.tensor.matmul(
                        ps,
                        lhsT=attnT_sb[:, p, qt * 128 : (qt + 1) * 128],
                        rhs=wo_sb[:, p, nt * 512 : (nt + 1) * 512],
                        start=(p == 0),
                        stop=(p == 1),
                    )
                nc.vector.tensor_copy(o_sb[:, nt * 512 : (nt + 1) * 512], ps)
            nc.sync.dma_start(out=out[qt * 128 : (qt + 1) * 128, :], in_=o_sb)

    def on(ph):
        return phases is None or ph in phases

    if phases is not None:
        # bisection mode: simple phase ordering
        if on("qk"):
            emit_qk(0)
        if on("v"):
            emit_v()
        if on("attn"):
            for qb in range(NQB):
                emit_attn(0, qb)
        if on("qk"):
            emit_qk(1)
        if on("attn"):
            for qb in range(NQB):
                emit_attn(1, qb)
        if on("wo"):
            emit_wo_part(range(NKT))
    else:
        # pipelined ordering: V and the following blocks' QK projections
        # hide under pair-0 attention, Wo under pair-1 attention
        emit_qk_part(0, 0)
        for qb in range(NQB):
            emit_v(range(4 * qb, 4 * qb + 4))
            hooks = [lambda q=qb: emit_qk_part(1, q)]
            if qb + 1 < NQB:
                hooks.append(lambda q=qb + 1: emit_qk_part(0, q))
            emit_attn(0, qb, prenorm=hooks)
        for qb, wo_qts in ((1, range(4, 8)), (2, range(8, 12)),
                           (3, range(12, 16)), (0, range(0, 4))):
            emit_attn(1, qb)
            emit_wo_part(wo_qts)
    ctx.close()


def make_tri():
    i = np.arange(128)[:, None]
    j = np.arange(128)[None, :]
    m = (i <= j).astype(np.float16)
    return np.concatenate([m, m], axis=1)  # duplicated per head pair


def make_in_maps(x, Wq, bq, Wk, bk, Wv, bv, Wo):
    tri = make_tri()
    eye = np.eye(128, dtype=np.float16)
    in_maps = []
    xTb8 = [np.ascontiguousarray(x[b].T.astype(NP8)) for b in range(2)]
    xTb16 = [np.ascontiguousarray(x[b].T.astype(np.float16)) for b in range(2)]
    for c in range(8):
        b, hg = divmod(c, 4)
        sl = slice(hg * S, (hg + 1) * S)
        in_maps.append(
            {
                "x8": xTb8[b],
                "xv": xTb16[b],
                "wq": np.ascontiguousarray(Wq[:, sl].astype(NP8)),
                "wk": np.ascontiguousarray(Wk[:, sl].astype(NP8)),
                "wv": np.ascontiguousarray(Wv[:, sl].astype(np.float16)),
                "wo": np.ascontiguousarray(Wo[sl, :].astype(np.float16)),
                "bq": np.ascontiguousarray(bq[sl].astype(np.float32).reshape(S, 1)),
                "bv": np.ascontiguousarray(bv[sl].astype(np.float32).reshape(1, S)),
                "tri": tri,
                "eye": eye,
            }
        )
    return in_maps


_NC_CACHE = None


def _get_nc():
    global _NC_CACHE
    if _NC_CACHE is None:
        _NC_CACHE = build_nc()
    return _NC_CACHE


def _run(x, Wq, bq, Wk, bk, Wv, bv, Wo, bo, trace=False, **spmd_kwargs):
    nc = _get_nc()
    in_maps = make_in_maps(
        np.asarray(x), np.asarray(Wq), np.asarray(bq), np.asarray(Wk),
        np.asarray(bk), np.asarray(Wv), np.asarray(bv), np.asarray(Wo),
    )
    res = run_bass_kernel_spmd(
        nc, in_maps, core_ids=list(range(8)), trace=trace, **spmd_kwargs
    )
    out = np.zeros((2, T, E), dtype=np.float32)
    for c in range(8):
        out[c // 4] += res.results[c]["out"]
    out += np.asarray(bo, dtype=np.float32)[None, None, :]
    return out, res


def kernel(x, Wq, bq, Wk, bk, Wv, bv, Wo, bo):
    out, _ = _run(x, Wq, bq, Wk, bk, Wv, bv, Wo, bo)
    return out
